# revision 8
# baseline (speedup 1.0000x reference)
"""Trainium2 Bass kernel for shifted-window correlation (27 shifts) + SE gate.

Reference (B=1, C=16, W=80, H=96, D=112):
  corr[w,h,d,k] = mean_c x1[c,w,h,d] * x2[c, w+sx, h+sy, d+sz]   (zero-padded)
  s = mean_{w,h,d} corr;  g = sigmoid(w1 @ relu(w0 @ s + b0) + b1)
  out = corr * g

Strategy (8 cores, W sharded 10/core). PE matmul cadence is ~165ns +
0.417ns/col regardless of weight reuse, so the selection-matmul reduction
floor is ~312us/core for 27 shifts; to beat it, the 5 sz=0 shifts in B_SET
run entirely on DVE in a d-partitioned layout (product + c tree-add, all
bf16 2x mode), while the remaining 22 shifts use the baseline path:
  - A-path: SBUF partition dim (c:16, h8:8); sy via free-dim h-halo rows
    of a single x2 window copy (no parity copies: misaligned strided
    operands still run DVE 2x); sx/sz as free-dim offsets. Products on
    DVE batched 3 h-rows per op; c-reduction on PE via block-diag
    selection matmuls (16 shifts in tile A rows 0:128, 6 in tile B rows
    0:48); ACT drains PSUM->SBUF with accum_out capturing squeeze
    partials; ungated corr spilled to DRAM.
  - B-path (k in B_SET, all sz=0): partition dim d:112; free (c,h,w);
    product [112,16,24,10] per h-chunk + 4 tree adds -> corrB in SBUF;
    squeeze via ACT accum + ones-matmul, rides the same allreduce.
  - Squeeze: per-core partials -> AllReduce (181 floats) -> on-device
    gate MLP; B-gates built as a [1,5] row then broadcast via matmul.
  - Phase 2: A re-reads spill (prefetch hides in collective latency),
    multiplies by per-partition gate; B gates corrB from SBUF.
"""

import sys
import types

import numpy as np
import ml_dtypes


def _install_ntff_hook_shim():
    """agent image's antenv lacks axon_hooks; needed only for trace=True."""
    if "antenv.axon_hooks" in sys.modules:
        return
    try:
        import antenv
        from trn_agent_boot.trn_boot import _ntff_profile_via_ctypes

        hook = _ntff_profile_via_ctypes("/opt/axon/libaxon_pjrt.so")
        mod = types.ModuleType("antenv.axon_hooks")
        ref = {"h": hook}
        mod.get_axon_ntff_profile_hook = lambda: ref["h"]
        mod.set_axon_ntff_profile_hook = lambda h: ref.__setitem__("h", h)
        sys.modules["antenv.axon_hooks"] = mod
        antenv.axon_hooks = mod
    except Exception:
        pass


_install_ntff_hook_shim()

import concourse.bacc as bacc  # noqa: E402
import concourse.tile as tile  # noqa: E402
import concourse.mybir as mybir  # noqa: E402
from concourse.bass_utils import run_bass_kernel_spmd  # noqa: E402

BF16 = mybir.dt.bfloat16
FP32 = mybir.dt.float32
AF = mybir.ActivationFunctionType
ALU = mybir.AluOpType

N_CORES = 8
C = 16
H8 = 8          # partition sub-dim over h (A-path)
K = 27
MID = 6

# shift order matches reference: k = dx*9 + dy*3 + dz, s* = d*-1
SHIFTS = [(dx - 1, dy - 1, dz - 1)
          for dx in range(3) for dy in range(3) for dz in range(3)]

B_SET = [13, 19, 25]                    # sz==0 shifts moved to the DVE path
NB = len(B_SET)
PE_SHIFTS = [k for k in range(27) if k not in B_SET]   # 22
NPE_A = 16                               # shifts in PE tile A (rows 0:128)
NPE_B = len(PE_SHIFTS) - NPE_A           # 6 shifts in tile B (rows 0:48)
RB = 8 * NPE_B                           # 48 spill rows for tile B
# chain length per 32-row group in tile B (g32 0: 4 shifts, g32 1: 2)
NV_B = [min(4, NPE_B - 4 * g) for g in range((NPE_B + 3) // 4)]


class Cfg:
    def __init__(self, W=80, H=96, D=112, hblk_t=3, w_sl=4):
        assert H % H8 == 0
        self.W, self.H, self.D = W, H, D
        self.Wc = W // N_CORES          # w columns per core
        self.HB = H // H8               # hblk extent (free dim)
        assert self.HB % hblk_t == 0
        self.hblk_t = hblk_t            # hblk rows per chunk
        self.n_chunks = self.HB // hblk_t
        self.w_sl = w_sl                # w per matmul slice
        self.n_ws = (self.Wc + w_sl - 1) // w_sl
        self.ch = H // 4                # B-path h-chunk


def _row_of(k, h8):
    """Spill partition row for PE shift k at h8."""
    i = PE_SHIFTS.index(k)
    if i < NPE_A:
        return 32 * (i // 4) + 8 * (i % 4) + h8
    j = i - NPE_A
    return 128 + 32 * (j // 4) + 8 * (j % 4) + h8


def build_nc(cfg: Cfg):
    nc = bacc.Bacc("TRN2", target_bir_lowering=False, debug=False,
                   num_devices=N_CORES)
    HB, Wc, D, H = cfg.HB, cfg.Wc, cfg.D, cfg.H
    t = cfg.hblk_t
    ch = H // cfg.n_chunks               # B-path h-chunk, tracks A-chunks
    RSP = 128 + RB                       # spill rows

    x1_d = nc.dram_tensor("x1", [128, HB, Wc, D], BF16, kind="ExternalInput")
    x2_d = nc.dram_tensor("x2a", [128, HB + 2, Wc + 2, D + 2], BF16,
                          kind="ExternalInput")
    x1b_d = nc.dram_tensor("x1b", [D, C, H, Wc], BF16, kind="ExternalInput")
    x2b_d = nc.dram_tensor("x2b", [D, C, H + 2, Wc + 2], BF16,
                           kind="ExternalInput")
    sel_d = nc.dram_tensor("selmats", [128, 128], BF16, kind="ExternalInput")
    w0a_d = nc.dram_tensor("w0a", [128, MID], FP32, kind="ExternalInput")
    w0b_d = nc.dram_tensor("w0b", [RB + NB, MID], FP32, kind="ExternalInput")
    w1a_d = nc.dram_tensor("w1ra", [MID, 128], FP32, kind="ExternalInput")
    w1b_d = nc.dram_tensor("w1rb", [MID, RB], FP32, kind="ExternalInput")
    w15_d = nc.dram_tensor("w15", [MID, NB], FP32, kind="ExternalInput")
    b15_d = nc.dram_tensor("b15", [1, NB], FP32, kind="ExternalInput")
    b0_d = nc.dram_tensor("b0c", [MID, 1], FP32, kind="ExternalInput")
    b1a_d = nc.dram_tensor("b1ra", [128, 1], FP32, kind="ExternalInput")
    b1b_d = nc.dram_tensor("b1rb", [RB, 1], FP32, kind="ExternalInput")
    ones_c_d = nc.dram_tensor("ones_c", [D, 1], FP32, kind="ExternalInput")
    ones_r_d = nc.dram_tensor("ones_r", [1, D], FP32, kind="ExternalInput")
    out_d = nc.dram_tensor("out", [RSP, HB, Wc, D], BF16, kind="ExternalOutput")
    outb_d = nc.dram_tensor("outb", [D, NB, H, Wc], BF16, kind="ExternalOutput")

    n_drain = cfg.n_chunks * t * cfg.n_ws
    CCR = 128 + RB + NB                  # allreduce payload rows

    with tile.TileContext(nc) as tc:
        with (
            tc.tile_pool(name="const", bufs=1) as cpool,
            tc.tile_pool(name="ps", bufs=1, space="PSUM") as ps,
            tc.tile_pool(name="dram", bufs=1, space="DRAM") as dram,
            tc.tile_pool(name="x2p", bufs=1) as x2pool,
            tc.tile_pool(name="pp", bufs=1) as ppool,
            tc.tile_pool(name="bb", bufs=1) as bpool,
            tc.tile_pool(name="stage", bufs=4) as spool,
            tc.tile_pool(name="ph2", bufs=2) as p2pool,
        ):
            x1t = cpool.tile([128, HB, Wc, D], BF16)
            selt = cpool.tile([128, 128], BF16)
            w0at = cpool.tile([128, MID], FP32)
            w0bt = cpool.tile([RB + NB, MID], FP32)
            w1at = cpool.tile([MID, 128], FP32)
            w1bt = cpool.tile([MID, RB], FP32)
            w15t = cpool.tile([MID, NB], FP32)
            b15t = cpool.tile([1, NB], FP32)
            b0t = cpool.tile([MID, 1], FP32)
            b1at = cpool.tile([128, 1], FP32)
            b1bt = cpool.tile([RB, 1], FP32)
            onesct = cpool.tile([D, 1], FP32)
            onesrt = cpool.tile([1, D], FP32)
            accA = cpool.tile([128, n_drain], FP32)
            accB = cpool.tile([RB, n_drain], FP32)
            corrB = cpool.tile([D, NB, H, Wc], BF16)
            pBd = cpool.tile([D, NB], FP32)
            scrB = cpool.tile([D, H, Wc], BF16)

            nc.sync.dma_start(x1t[:], x1_d[:])
            nc.sync.dma_start(selt[:], sel_d[:])
            nc.sync.dma_start(w0at[:], w0a_d[:])
            nc.sync.dma_start(w0bt[:], w0b_d[:])
            nc.sync.dma_start(w1at[:], w1a_d[:])
            nc.sync.dma_start(w1bt[:], w1b_d[:])
            nc.sync.dma_start(w15t[:], w15_d[:])
            nc.sync.dma_start(b15t[:], b15_d[:])
            nc.sync.dma_start(b0t[:], b0_d[:])
            nc.sync.dma_start(b1at[:], b1a_d[:])
            nc.sync.dma_start(b1bt[:], b1b_d[:])
            nc.sync.dma_start(onesct[:], ones_c_d[:])
            nc.sync.dma_start(onesrt[:], ones_r_d[:])

            spill = dram.tile([RSP, HB, Wc, D], BF16)

            # Warm-up collective: absorbs launch skew + CC firmware setup.
            warm_in = dram.tile([MID, 1], FP32)
            warm_out = dram.tile([MID, 1], FP32)
            nc.sync.dma_start(warm_in[:], b0_d[:])
            nc.gpsimd.collective_compute(
                "AllReduce", ALU.add,
                replica_groups=[list(range(N_CORES))],
                ins=[warm_in[:].opt()],
                outs=[warm_out[:].opt()],
            )

            early_drains = (cfg.n_chunks - 1) * t * cfg.n_ws
            pA1 = cpool.tile([128, 1], FP32)
            pB1 = cpool.tile([RB, 1], FP32)
            cc1_in = dram.tile([128 + RB, 1], FP32)
            cc1_out = dram.tile([128 + RB, 1], FP32)

            drain_i = 0
            for chk in range(cfg.n_chunks):
                hb0 = chk * t

                if chk == cfg.n_chunks - 1:
                    # early allreduce over chunks 0..n-2 A-partials
                    nc.vector.tensor_reduce(
                        pA1[:], accA[:, 0:early_drains],
                        mybir.AxisListType.X, ALU.add)
                    nc.vector.tensor_reduce(
                        pB1[:], accB[:, 0:early_drains],
                        mybir.AxisListType.X, ALU.add)
                    nc.sync.dma_start(cc1_in[0:128, :], pA1[:])
                    nc.sync.dma_start(cc1_in[128:128 + RB, :], pB1[:])
                    nc.gpsimd.collective_compute(
                        "AllReduce", ALU.add,
                        replica_groups=[list(range(N_CORES))],
                        ins=[cc1_in[:].opt()],
                        outs=[cc1_out[:].opt()],
                    )

                # x2 window for this chunk: rows hb0..hb0+t+2 of halo'd copy
                x2t = x2pool.tile([128, t + 2, Wc + 2, D + 2], BF16,
                                  tag="x2", bufs=2, name="x2w")
                nc.sync.dma_start(x2t[:], x2_d[:, hb0:hb0 + t + 2, :, :])

                psA = [ps.tile([128, min(cfg.w_sl, Wc - ws * cfg.w_sl) * D],
                               FP32, tag=f"psA{ws}", name=f"psA{ws}",
                               padded_shape=[128, 512])
                       for ws in range(cfg.n_ws)]
                psB = [ps.tile([128, min(cfg.w_sl, Wc - ws * cfg.w_sl) * D],
                               FP32, tag=f"psB{ws}", name=f"psB{ws}",
                               padded_shape=[128, 512])
                       for ws in range(cfg.n_ws)]

                # B-path chunk inputs (h rows chk*ch..); the per-shift
                # product+tree units are interleaved between A rows below
                # to fill DVE slack while the PE chews A matmuls.
                h0 = chk * ch
                x1bc = bpool.tile([D, C, ch, Wc], BF16, tag="x1b", bufs=2)
                x2bc = bpool.tile([D, C, ch + 2, Wc + 2], BF16,
                                  tag="x2b", bufs=2)
                nc.sync.dma_start(x1bc[:], x1b_d[:, :, h0:h0 + ch, :])
                nc.sync.dma_start(x2bc[:], x2b_d[:, :, h0:h0 + ch + 2, :])

                def b_unit(ki):
                    k = B_SET[ki]
                    sx, sy, _ = SHIFTS[k]
                    pb = bpool.tile([D, C, ch, Wc], BF16, tag="pb", bufs=2)
                    nc.vector.tensor_tensor(
                        pb[:], x1bc[:],
                        x2bc[:, :, 1 + sy:1 + sy + ch, 1 + sx:1 + sx + Wc],
                        ALU.mult)
                    t1 = bpool.tile([D, 8, ch, Wc], BF16, tag="t1", bufs=2)
                    nc.vector.tensor_tensor(
                        t1[:], pb[:, 0:8], pb[:, 8:16], ALU.add)
                    t2 = bpool.tile([D, 4, ch, Wc], BF16, tag="t2", bufs=2)
                    nc.vector.tensor_tensor(
                        t2[:], t1[:, 0:4], t1[:, 4:8], ALU.add)
                    t3 = bpool.tile([D, 2, ch, Wc], BF16, tag="t3", bufs=2)
                    nc.vector.tensor_tensor(
                        t3[:], t2[:, 0:2], t2[:, 2:4], ALU.add)
                    nc.vector.tensor_tensor(
                        corrB[:, ki, h0:h0 + ch, :],
                        t3[:, 0], t3[:, 1], ALU.add)

                bq = list(range(NB))    # B-units left to emit this chunk
                for j in range(t):
                    for i, k in enumerate(PE_SHIFTS):
                        sx, sy, sz = SHIFTS[k]
                        p = ppool.tile([128, Wc, D], BF16, tag="P", bufs=6)
                        nc.vector.tensor_tensor(
                            p[:],
                            x1t[:, hb0 + j, :, :],
                            x2t[:, 1 + sy + j, 1 + sx:1 + sx + Wc,
                                1 + sz:1 + sz + D],
                            ALU.mult,
                        )
                        if i < NPE_A:
                            g32, v, nv, pst = i // 4, i % 4, 4, psA
                        else:
                            jj = i - NPE_A
                            g32, v = jj // 4, jj % 4
                            nv, pst = NV_B[g32], psB
                        for ws in range(cfg.n_ws):
                            w0 = ws * cfg.w_sl
                            nw = min(cfg.w_sl, Wc - w0)
                            nc.tensor.matmul(
                                pst[ws][32 * g32:32 * g32 + 32, :],
                                selt[:, 32 * v:32 * v + 32],
                                p[:, w0:w0 + nw, :],
                                start=(v == 0), stop=(v == nv - 1),
                                tile_position=(0, 32 * g32),
                            )
                        if i == 10 and bq:
                            b_unit(bq.pop(0))
                    if j == t - 1 and bq:
                        while bq:
                            b_unit(bq.pop(0))
                    for ws in range(cfg.n_ws):
                        w0 = ws * cfg.w_sl
                        nw = min(cfg.w_sl, Wc - w0)
                        nfree = nw * D
                        stA = spool.tile([128, nfree], BF16, tag="stA")
                        stB = spool.tile([RB, nfree], BF16, tag="stB")
                        nc.scalar.activation(
                            stA[:], psA[ws][:, 0:nfree], AF.Copy,
                            accum_out=accA[:, drain_i:drain_i + 1])
                        nc.scalar.activation(
                            stB[:], psB[ws][0:RB, 0:nfree], AF.Copy,
                            accum_out=accB[:, drain_i:drain_i + 1])
                        nc.sync.dma_start(
                            spill[0:128, hb0 + j, w0:w0 + nw, :], stA[:])
                        nc.sync.dma_start(
                            spill[128:RSP, hb0 + j, w0:w0 + nw, :], stB[:])
                        drain_i += 1

            # ---- B-path squeeze: per-shift spatial sums, then sum over d ----
            for ki in range(NB):
                nc.scalar.activation(
                    scrB[:], corrB[:, ki, :, :], AF.Copy,
                    accum_out=pBd[:, ki:ki + 1])
            psq = ps.tile([1, NB], FP32, tag="psA0", padded_shape=[128, 512])
            nc.tensor.matmul(psq[:], onesct[:], pBd[:], start=True, stop=True)
            sBt = cpool.tile([1, NB], FP32)
            nc.scalar.activation(sBt[:], psq[:], AF.Copy)

            # ---- last-chunk partials + final allreduce + gate MLP ----
            pA = cpool.tile([128, 1], FP32)
            pB = cpool.tile([RB, 1], FP32)
            scrA = cpool.tile([128, n_drain - early_drains], FP32)
            scrB2 = cpool.tile([RB, n_drain - early_drains], FP32)
            nc.scalar.activation(scrA[:], accA[:, early_drains:n_drain],
                                 AF.Copy, accum_out=pA[:])
            nc.scalar.activation(scrB2[:], accB[:, early_drains:n_drain],
                                 AF.Copy, accum_out=pB[:])
            pAg1 = cpool.tile([128, 1], FP32)
            pBg1 = cpool.tile([RB, 1], FP32)
            nc.sync.dma_start(pAg1[:], cc1_out[0:128, :])
            nc.sync.dma_start(pBg1[:], cc1_out[128:128 + RB, :])
            ccA = cpool.tile([128, 1], FP32)
            ccB = cpool.tile([RB, 1], FP32)
            nc.scalar.activation(ccA[:], pAg1[:], AF.Identity,
                                 bias=pA[:], scale=1.0 / N_CORES)
            nc.scalar.activation(ccB[:], pBg1[:], AF.Identity,
                                 bias=pB[:], scale=1.0 / N_CORES)
            cc_in = dram.tile([CCR, 1], FP32)
            cc_out = dram.tile([CCR, 1], FP32)
            nc.sync.dma_start(cc_in[0:128, :], ccA[:])
            nc.sync.dma_start(cc_in[128:128 + RB, :], ccB[:])
            nc.sync.dma_start(cc_in[128 + RB:CCR, :], sBt[:])
            nc.gpsimd.collective_compute(
                "AllReduce", ALU.add,
                replica_groups=[list(range(N_CORES))],
                ins=[cc_in[:].opt()],
                outs=[cc_out[:].opt()],
            )
            pAg = cpool.tile([128, 1], FP32)
            pBg = cpool.tile([RB + NB, 1], FP32)
            nc.sync.dma_start(pAg[:], cc_out[0:128, :])
            nc.sync.dma_start(pBg[:], cc_out[128:CCR, :])

            hps = ps.tile([MID, 1], FP32, tag="psA1", padded_shape=[128, 512])
            nc.tensor.matmul(hps[:], w0at[:], pAg[:], start=True, stop=False)
            nc.tensor.matmul(hps[:], w0bt[:], pBg[:], start=False, stop=True)
            hvec = cpool.tile([MID, 1], FP32)
            nc.scalar.activation(hvec[:], hps[:], AF.Relu, bias=b0t[:], scale=1.0)
            gpsA = ps.tile([128, 1], FP32, tag="psA2", padded_shape=[128, 512])
            gpsB = ps.tile([RB, 1], FP32, tag="psB0", padded_shape=[128, 512])
            nc.tensor.matmul(gpsA[:], w1at[:], hvec[:], start=True, stop=True)
            nc.tensor.matmul(gpsB[:], w1bt[:], hvec[:], start=True, stop=True)
            gA = cpool.tile([128, 1], FP32)
            gB = cpool.tile([RB, 1], FP32)
            nc.scalar.activation(gA[:], gpsA[:], AF.Sigmoid, bias=b1at[:], scale=1.0)
            nc.scalar.activation(gB[:], gpsB[:], AF.Sigmoid, bias=b1bt[:], scale=1.0)

            # B-gates: [1,NB] row = sigmoid(hvec.T @ w15 + b15), broadcast to d
            g5ps = ps.tile([1, NB], FP32, tag="psB1", padded_shape=[128, 512])
            nc.tensor.matmul(g5ps[:], hvec[:], w15t[:], start=True, stop=True)
            g5s = cpool.tile([1, NB], FP32)
            nc.vector.tensor_tensor(g5s[:], g5ps[:], b15t[:], ALU.add)
            g5t = cpool.tile([1, NB], FP32)
            nc.scalar.activation(g5t[:], g5s[:], AF.Sigmoid)
            gbps = ps.tile([D, NB], FP32, tag="psB2", padded_shape=[128, 512])
            nc.tensor.matmul(gbps[:], onesrt[:], g5t[:], start=True, stop=True)
            gbc = cpool.tile([D, NB], FP32)
            nc.scalar.activation(gbc[:], gbps[:], AF.Copy)

            # ---- phase 2: gated writeout ----
            for ki in range(NB):
                ob = p2pool.tile([D, H, Wc], BF16, tag="p2b", bufs=2)
                nc.vector.tensor_scalar(ob[:], corrB[:, ki, :, :],
                                        gbc[:, ki:ki + 1], None, ALU.mult)
                nc.scalar.dma_start(outb_d[:, ki, :, :], ob[:])
            assert HB % 2 == 0
            for hb in range(0, HB, 2):
                stA2i = p2pool.tile([128, 2, Wc, D], BF16, tag="p2ai", bufs=3)
                stA2o = p2pool.tile([128, 2, Wc, D], BF16, tag="p2ao", bufs=2)
                nc.sync.dma_start(stA2i[:], spill[0:128, hb:hb + 2, :, :])
                nc.vector.tensor_scalar(stA2o[:], stA2i[:], gA[:], None, ALU.mult)
                nc.scalar.dma_start(out_d[0:128, hb:hb + 2, :, :], stA2o[:])
                stB2i = p2pool.tile([RB, 2, Wc, D], BF16, tag="p2bi", bufs=3)
                stB2o = p2pool.tile([RB, 2, Wc, D], BF16, tag="p2bo", bufs=2)
                nc.sync.dma_start(stB2i[:], spill[128:RSP, hb:hb + 2, :, :])
                nc.vector.tensor_scalar(stB2o[:], stB2i[:], gB[:], None, ALU.mult)
                nc.scalar.dma_start(out_d[128:RSP, hb:hb + 2, :, :], stB2o[:])

    nc.compile()
    return nc


# ---------------- host-side prep / assembly ----------------

def make_gate_consts(w0, b0, w1, b1, cfg: Cfg):
    norm = 1.0 / (cfg.W * cfg.H * cfg.D)
    sel = np.zeros((128, 128), dtype=np.float32)
    for v in range(4):
        for c in range(C):
            for h8 in range(H8):
                sel[c * H8 + h8, 32 * v + 8 * v + h8] = 1.0 / 16
    w0 = np.asarray(w0, dtype=np.float32)
    w1 = np.asarray(w1, dtype=np.float32)
    b1 = np.asarray(b1, dtype=np.float32)
    w0a = np.zeros((128, MID), dtype=np.float32)
    w0b = np.zeros((RB + NB, MID), dtype=np.float32)
    w1ra = np.zeros((MID, 128), dtype=np.float32)
    w1rb = np.zeros((MID, RB), dtype=np.float32)
    b1ra = np.zeros((128, 1), dtype=np.float32)
    b1rb = np.zeros((RB, 1), dtype=np.float32)
    for k in PE_SHIFTS:
        for h8 in range(H8):
            r = _row_of(k, h8)
            if r < 128:
                w0a[r, :] = w0[:, k] * norm
                w1ra[:, r] = w1[k, :]
                b1ra[r, 0] = b1[k]
            else:
                w0b[r - 128, :] = w0[:, k] * norm
                w1rb[:, r - 128] = w1[k, :]
                b1rb[r - 128, 0] = b1[k]
    w15 = np.zeros((MID, NB), dtype=np.float32)
    b15 = np.zeros((1, NB), dtype=np.float32)
    for ki, k in enumerate(B_SET):
        w0b[RB + ki, :] = w0[:, k] * norm
        w15[:, ki] = w1[k, :]
        b15[0, ki] = b1[k]
    return {
        "selmats": sel.astype(ml_dtypes.bfloat16),
        "w0a": w0a, "w0b": w0b, "w1ra": w1ra, "w1rb": w1rb,
        "w15": w15, "b15": b15,
        "b0c": np.asarray(b0, dtype=np.float32).reshape(MID, 1),
        "b1ra": b1ra, "b1rb": b1rb,
        "ones_c": np.ones((cfg.D, 1), dtype=np.float32),
        "ones_r": np.ones((1, cfg.D), dtype=np.float32),
    }


def _fold(a, HB):
    # [C, w, H, D'] -> [(c h8), hblk, w, d]
    Cc, ww, hh, dd = a.shape
    a = a.reshape(Cc, ww, H8, HB, dd)
    a = np.ascontiguousarray(a.transpose(0, 2, 3, 1, 4))
    return a.reshape(C * H8, HB, ww, dd)


def _fold_halo(a, HB):
    # [C, w, Hp=H+2, D'] (padded h) -> [(c h8), HB+2, w, d] with h-halo rows
    Cc, ww, hp, dd = a.shape
    out = np.empty((Cc, H8, HB + 2, ww, dd), dtype=a.dtype)
    for h8 in range(H8):
        out[:, h8] = a[:, :, h8 * HB:h8 * HB + HB + 2, :].transpose(0, 2, 1, 3)
    return out.reshape(Cc * H8, HB + 2, ww, dd)


def make_inputs_per_core(x_1, x_2, w0, b0, w1, b1, cfg: Cfg):
    """x_1/x_2: [1, C, W, H, D] float32 -> list of per-core input dicts."""
    W, H, D = cfg.W, cfg.H, cfg.D
    Wc, HB = cfg.Wc, cfg.HB
    x1 = np.asarray(x_1)[0].astype(ml_dtypes.bfloat16)      # [C, W, H, D]
    x2 = np.asarray(x_2)[0].astype(ml_dtypes.bfloat16)
    x2p = np.zeros((C, W + 2, H + 2, D + 2), dtype=ml_dtypes.bfloat16)
    x2p[:, 1:W + 1, 1:H + 1, 1:D + 1] = x2

    consts = make_gate_consts(w0, b0, w1, b1, cfg)
    in_maps = []
    for ci in range(N_CORES):
        ws = ci * Wc
        m = dict(consts)
        m["x1"] = _fold(x1[:, ws:ws + Wc, :, :], HB)
        m["x2a"] = _fold_halo(x2p[:, ws:ws + Wc + 2, :, :], HB)
        m["x1b"] = np.ascontiguousarray(
            x1[:, ws:ws + Wc, :, :].transpose(3, 0, 2, 1)
        ).astype(np.float32).__mul__(1.0 / 16).astype(ml_dtypes.bfloat16)
        m["x2b"] = np.ascontiguousarray(
            x2p[:, ws:ws + Wc + 2, :, 1:1 + D].transpose(3, 0, 2, 1))
        in_maps.append(m)
    return in_maps


def assemble_output(results, cfg: Cfg):
    W, H, D = cfg.W, cfg.H, cfg.D
    Wc, HB = cfg.Wc, cfg.HB
    out = np.empty((W, H, D, K), dtype=np.float32)
    rows = np.empty((len(PE_SHIFTS), H8), dtype=np.int64)
    for i, k in enumerate(PE_SHIFTS):
        for h8 in range(H8):
            rows[i, h8] = _row_of(k, h8)
    for ci, r in enumerate(results):
        o = np.asarray(r["out"]).reshape(128 + RB, HB, Wc, D)
        core = o[rows]                        # [npe, H8, HB, Wc, D]
        core = core.transpose(3, 1, 2, 4, 0)  # [Wc, H8, HB, D, npe]
        core = core.reshape(Wc, H, D, len(PE_SHIFTS))
        for i, k in enumerate(PE_SHIFTS):
            out[ci * Wc:(ci + 1) * Wc, :, :, k] = core[..., i]
        ob = np.asarray(r["outb"]).reshape(D, NB, H, Wc)
        for ki, k in enumerate(B_SET):
            out[ci * Wc:(ci + 1) * Wc, :, :, k] = ob[:, ki].transpose(2, 1, 0)
    return out[None]


_CACHE = {}
TRACE = False           # test harness can set kernel.TRACE = True


def kernel(x_1, x_2, w0, b0, w1, b1):
    cfg = Cfg()
    if "nc" not in _CACHE:
        _CACHE["nc"] = build_nc(cfg)
    nc = _CACHE["nc"]
    in_maps = make_inputs_per_core(x_1, x_2, w0, b0, w1, b1, cfg)
    last_exc = None
    for _attempt in range(3):
        try:
            res = run_bass_kernel_spmd(nc, in_maps,
                                       core_ids=list(range(N_CORES)),
                                       trace=TRACE)
            break
        except Exception as e:  # transient NRT device errors: retry
            last_exc = e
    else:
        raise last_exc
    _CACHE["last_res"] = res
    return assemble_output(res.results, cfg)


# revision 10
# speedup vs baseline: 1.1296x; 1.1296x over previous
"""Trainium2 Bass kernel for shifted-window correlation (27 shifts) + SE gate.

Reference (B=1, C=16, W=80, H=96, D=112):
  corr[w,h,d,k] = mean_c x1[c,w,h,d] * x2[c, w+sx, h+sy, d+sz]   (zero-padded)
  s = mean_{w,h,d} corr;  g = sigmoid(w1 @ relu(w0 @ s + b0) + b1)
  out = corr * g

Strategy (8 cores, W sharded 10/core). PE matmul cadence is ~165ns +
0.417ns/col regardless of weight reuse, so the selection-matmul reduction
floor is ~312us/core for 27 shifts; to beat it, the 5 sz=0 shifts in B_SET
run entirely on DVE in a d-partitioned layout (product + c tree-add, all
bf16 2x mode), while the remaining 22 shifts use the baseline path:
  - A-path: SBUF partition dim (c:16, h8:8); sy via free-dim h-halo rows
    of a single x2 window copy (no parity copies: misaligned strided
    operands still run DVE 2x); sx/sz as free-dim offsets. Products on
    DVE batched 3 h-rows per op; c-reduction on PE via block-diag
    selection matmuls (16 shifts in tile A rows 0:128, 6 in tile B rows
    0:48); ACT drains PSUM->SBUF with accum_out capturing squeeze
    partials; ungated corr spilled to DRAM.
  - B-path (k in B_SET, all sz=0): partition dim d:112; free (c,h,w);
    product [112,16,24,10] per h-chunk + 4 tree adds -> corrB in SBUF;
    squeeze via ACT accum + ones-matmul, rides the same allreduce.
  - Squeeze: per-core partials -> AllReduce (181 floats) -> on-device
    gate MLP; B-gates built as a [1,5] row then broadcast via matmul.
  - Phase 2: A re-reads spill (prefetch hides in collective latency),
    multiplies by per-partition gate; B gates corrB from SBUF.
"""

import sys
import types

import numpy as np
import ml_dtypes


def _install_ntff_hook_shim():
    """agent image's antenv lacks axon_hooks; needed only for trace=True."""
    if "antenv.axon_hooks" in sys.modules:
        return
    try:
        import antenv
        from trn_agent_boot.trn_boot import _ntff_profile_via_ctypes

        hook = _ntff_profile_via_ctypes("/opt/axon/libaxon_pjrt.so")
        mod = types.ModuleType("antenv.axon_hooks")
        ref = {"h": hook}
        mod.get_axon_ntff_profile_hook = lambda: ref["h"]
        mod.set_axon_ntff_profile_hook = lambda h: ref.__setitem__("h", h)
        sys.modules["antenv.axon_hooks"] = mod
        antenv.axon_hooks = mod
    except Exception:
        pass


_install_ntff_hook_shim()

import concourse.bacc as bacc  # noqa: E402
import concourse.tile as tile  # noqa: E402
import concourse.mybir as mybir  # noqa: E402
from concourse.bass_utils import run_bass_kernel_spmd  # noqa: E402

BF16 = mybir.dt.bfloat16
FP32 = mybir.dt.float32
AF = mybir.ActivationFunctionType
ALU = mybir.AluOpType

N_CORES = 8
C = 16
H8 = 8          # partition sub-dim over h (A-path)
K = 27
MID = 6

# shift order matches reference: k = dx*9 + dy*3 + dz, s* = d*-1
SHIFTS = [(dx - 1, dy - 1, dz - 1)
          for dx in range(3) for dy in range(3) for dz in range(3)]

B_SET = [13, 19, 25]                    # sz==0 shifts moved to the DVE path
NB = len(B_SET)
PE_SHIFTS = [k for k in range(27) if k not in B_SET]   # 22
NPE_A = 16                               # shifts in PE tile A (rows 0:128)
NPE_B = len(PE_SHIFTS) - NPE_A           # 6 shifts in tile B (rows 0:48)
RB = 8 * NPE_B                           # 48 spill rows for tile B
# chain length per 32-row group in tile B (g32 0: 4 shifts, g32 1: 2)
NV_B = [min(4, NPE_B - 4 * g) for g in range((NPE_B + 3) // 4)]


class Cfg:
    def __init__(self, W=80, H=96, D=112, hblk_t=3, w_sl=4):
        assert H % H8 == 0
        self.W, self.H, self.D = W, H, D
        self.Wc = W // N_CORES          # w columns per core
        self.HB = H // H8               # hblk extent (free dim)
        assert self.HB % hblk_t == 0
        self.hblk_t = hblk_t            # hblk rows per chunk
        self.n_chunks = self.HB // hblk_t
        self.w_sl = w_sl                # w per matmul slice
        self.n_ws = (self.Wc + w_sl - 1) // w_sl
        self.ch = H // 4                # B-path h-chunk


def _row_of(k, h8):
    """Spill partition row for PE shift k at h8."""
    i = PE_SHIFTS.index(k)
    if i < NPE_A:
        return 32 * (i // 4) + 8 * (i % 4) + h8
    j = i - NPE_A
    return 128 + 32 * (j // 4) + 8 * (j % 4) + h8


def build_nc(cfg: Cfg):
    nc = bacc.Bacc("TRN2", target_bir_lowering=False, debug=False,
                   num_devices=N_CORES)
    HB, Wc, D, H = cfg.HB, cfg.Wc, cfg.D, cfg.H
    t = cfg.hblk_t
    ch = H // cfg.n_chunks               # B-path h-chunk, tracks A-chunks
    RSP = 128 + RB                       # spill rows

    x1_d = nc.dram_tensor("x1", [128, HB, Wc, D], BF16, kind="ExternalInput")
    x2_d = nc.dram_tensor("x2a", [128, HB + 2, Wc + 2, D + 2], BF16,
                          kind="ExternalInput")
    x1b_d = nc.dram_tensor("x1b", [D, C, H, Wc], BF16, kind="ExternalInput")
    x2b_d = nc.dram_tensor("x2b", [D, C, H + 2, Wc + 2], BF16,
                           kind="ExternalInput")
    sel_d = nc.dram_tensor("selmats", [128, 128], BF16, kind="ExternalInput")
    w0a_d = nc.dram_tensor("w0a", [128, MID], FP32, kind="ExternalInput")
    w0b_d = nc.dram_tensor("w0b", [RB + NB, MID], FP32, kind="ExternalInput")
    w1a_d = nc.dram_tensor("w1ra", [MID, 128], FP32, kind="ExternalInput")
    w1b_d = nc.dram_tensor("w1rb", [MID, RB], FP32, kind="ExternalInput")
    w15_d = nc.dram_tensor("w15", [MID, NB], FP32, kind="ExternalInput")
    b15_d = nc.dram_tensor("b15", [1, NB], FP32, kind="ExternalInput")
    b0_d = nc.dram_tensor("b0c", [MID, 1], FP32, kind="ExternalInput")
    b1a_d = nc.dram_tensor("b1ra", [128, 1], FP32, kind="ExternalInput")
    b1b_d = nc.dram_tensor("b1rb", [RB, 1], FP32, kind="ExternalInput")
    ones_c_d = nc.dram_tensor("ones_c", [D, 1], FP32, kind="ExternalInput")
    ones_r_d = nc.dram_tensor("ones_r", [1, D], FP32, kind="ExternalInput")
    out_d = nc.dram_tensor("out", [RSP, HB, Wc, D], BF16, kind="ExternalOutput")
    outb_d = nc.dram_tensor("outb", [D, NB, H, Wc], BF16, kind="ExternalOutput")

    n_drain = cfg.n_chunks * t * cfg.n_ws
    CCR = 128 + RB + NB                  # allreduce payload rows

    with tile.TileContext(nc) as tc:
        with (
            tc.tile_pool(name="const", bufs=1) as cpool,
            tc.tile_pool(name="ps", bufs=1, space="PSUM") as ps,
            tc.tile_pool(name="dram", bufs=1, space="DRAM") as dram,
            tc.tile_pool(name="x2p", bufs=1) as x2pool,
            tc.tile_pool(name="pp", bufs=1) as ppool,
            tc.tile_pool(name="bb", bufs=1) as bpool,
            tc.tile_pool(name="stage", bufs=4) as spool,
            tc.tile_pool(name="ph2", bufs=2) as p2pool,
        ):
            x1t = cpool.tile([128, HB, Wc, D], BF16)
            selt = cpool.tile([128, 128], BF16)
            w0at = cpool.tile([128, MID], FP32)
            w0bt = cpool.tile([RB + NB, MID], FP32)
            w1at = cpool.tile([MID, 128], FP32)
            w1bt = cpool.tile([MID, RB], FP32)
            w15t = cpool.tile([MID, NB], FP32)
            b15t = cpool.tile([1, NB], FP32)
            b0t = cpool.tile([MID, 1], FP32)
            b1at = cpool.tile([128, 1], FP32)
            b1bt = cpool.tile([RB, 1], FP32)
            onesct = cpool.tile([D, 1], FP32)
            onesrt = cpool.tile([1, D], FP32)
            accA = cpool.tile([128, n_drain], FP32)
            accB = cpool.tile([RB, n_drain], FP32)
            corrB = cpool.tile([D, NB, H, Wc], BF16)
            pBd = cpool.tile([D, NB], FP32)
            scrB = cpool.tile([D, H, Wc], BF16)

            nc.sync.dma_start(x1t[:], x1_d[:])
            nc.sync.dma_start(selt[:], sel_d[:])
            nc.sync.dma_start(w0at[:], w0a_d[:])
            nc.sync.dma_start(w0bt[:], w0b_d[:])
            nc.sync.dma_start(w1at[:], w1a_d[:])
            nc.sync.dma_start(w1bt[:], w1b_d[:])
            nc.sync.dma_start(w15t[:], w15_d[:])
            nc.sync.dma_start(b15t[:], b15_d[:])
            nc.sync.dma_start(b0t[:], b0_d[:])
            nc.sync.dma_start(b1at[:], b1a_d[:])
            nc.sync.dma_start(b1bt[:], b1b_d[:])
            nc.sync.dma_start(onesct[:], ones_c_d[:])
            nc.sync.dma_start(onesrt[:], ones_r_d[:])

            spill = dram.tile([RSP, HB, Wc, D], BF16)

            # Warm-up collective: absorbs launch skew + CC firmware setup.
            warm_in = dram.tile([MID, 1], FP32)
            warm_out = dram.tile([MID, 1], FP32)
            nc.sync.dma_start(warm_in[:], b0_d[:])
            nc.gpsimd.collective_compute(
                "AllReduce", ALU.add,
                replica_groups=[list(range(N_CORES))],
                ins=[warm_in[:].opt()],
                outs=[warm_out[:].opt()],
            )

            early_drains = (cfg.n_chunks - 1) * t * cfg.n_ws
            pA1 = cpool.tile([128, 1], FP32)
            pB1 = cpool.tile([RB, 1], FP32)
            cc1_in = dram.tile([128 + RB, 1], FP32)
            cc1_out = dram.tile([128 + RB, 1], FP32)

            drain_i = 0
            for chk in range(cfg.n_chunks):
                hb0 = chk * t

                if chk == cfg.n_chunks - 1:
                    # early allreduce over chunks 0..n-2 A-partials
                    nc.vector.tensor_reduce(
                        pA1[:], accA[:, 0:early_drains],
                        mybir.AxisListType.X, ALU.add)
                    nc.vector.tensor_reduce(
                        pB1[:], accB[:, 0:early_drains],
                        mybir.AxisListType.X, ALU.add)
                    nc.sync.dma_start(cc1_in[0:128, :], pA1[:])
                    nc.sync.dma_start(cc1_in[128:128 + RB, :], pB1[:])
                    nc.gpsimd.collective_compute(
                        "AllReduce", ALU.add,
                        replica_groups=[list(range(N_CORES))],
                        ins=[cc1_in[:].opt()],
                        outs=[cc1_out[:].opt()],
                    )

                # x2 window for this chunk: rows hb0..hb0+t+2 of halo'd copy
                x2t = x2pool.tile([128, t + 2, Wc + 2, D + 2], BF16,
                                  tag="x2", bufs=2, name="x2w")
                nc.sync.dma_start(x2t[:], x2_d[:, hb0:hb0 + t + 2, :, :])

                psA = [ps.tile([128, min(cfg.w_sl, Wc - ws * cfg.w_sl) * D],
                               FP32, tag=f"psA{ws}", name=f"psA{ws}",
                               padded_shape=[128, 512])
                       for ws in range(cfg.n_ws)]
                psB = [ps.tile([128, min(cfg.w_sl, Wc - ws * cfg.w_sl) * D],
                               FP32, tag=f"psB{ws}", name=f"psB{ws}",
                               padded_shape=[128, 512])
                       for ws in range(cfg.n_ws)]

                # B-path chunk inputs (h rows chk*ch..); the per-shift
                # product+tree units are interleaved between A rows below
                # to fill DVE slack while the PE chews A matmuls.
                h0 = chk * ch
                x1bc = bpool.tile([D, C, ch, Wc], BF16, tag="x1b", bufs=2)
                x2bc = bpool.tile([D, C, ch + 2, Wc + 2], BF16,
                                  tag="x2b", bufs=2)
                nc.sync.dma_start(x1bc[:], x1b_d[:, :, h0:h0 + ch, :])
                nc.sync.dma_start(x2bc[:], x2b_d[:, :, h0:h0 + ch + 2, :])

                def b_ops(ki):
                    # one B-shift as 5 separately-emittable DVE ops, so the
                    # A-product stream is never blocked longer than one op
                    k = B_SET[ki]
                    sx, sy, _ = SHIFTS[k]
                    st = {}

                    def op1():
                        st["pb"] = bpool.tile([D, C, ch, Wc], BF16,
                                              tag="pb", bufs=2, name="pb")
                        nc.vector.tensor_tensor(
                            st["pb"][:], x1bc[:],
                            x2bc[:, :, 1 + sy:1 + sy + ch,
                                 1 + sx:1 + sx + Wc],
                            ALU.mult)

                    def op2():
                        st["t1"] = bpool.tile([D, 8, ch, Wc], BF16,
                                              tag="t1", bufs=2, name="t1")
                        nc.vector.tensor_tensor(
                            st["t1"][:], st["pb"][:, 0:8], st["pb"][:, 8:16],
                            ALU.add)

                    def op3():
                        st["t2"] = bpool.tile([D, 4, ch, Wc], BF16,
                                              tag="t2", bufs=2, name="t2")
                        nc.vector.tensor_tensor(
                            st["t2"][:], st["t1"][:, 0:4], st["t1"][:, 4:8],
                            ALU.add)

                    def op4():
                        st["t3"] = bpool.tile([D, 2, ch, Wc], BF16,
                                              tag="t3", bufs=2, name="t3")
                        nc.vector.tensor_tensor(
                            st["t3"][:], st["t2"][:, 0:2], st["t2"][:, 2:4],
                            ALU.add)

                    def op5():
                        nc.vector.tensor_tensor(
                            corrB[:, ki, h0:h0 + ch, :],
                            st["t3"][:, 0], st["t3"][:, 1], ALU.add)

                    return [op1, op2, op3, op4, op5]

                bq = [op for ki in range(NB) for op in b_ops(ki)]
                for j in range(t):
                    for i, k in enumerate(PE_SHIFTS):
                        sx, sy, sz = SHIFTS[k]
                        p = ppool.tile([128, Wc, D], BF16, tag="P", bufs=6)
                        nc.vector.tensor_tensor(
                            p[:],
                            x1t[:, hb0 + j, :, :],
                            x2t[:, 1 + sy + j, 1 + sx:1 + sx + Wc,
                                1 + sz:1 + sz + D],
                            ALU.mult,
                        )
                        if i < NPE_A:
                            g32, v, nv, pst = i // 4, i % 4, 4, psA
                        else:
                            jj = i - NPE_A
                            g32, v = jj // 4, jj % 4
                            nv, pst = NV_B[g32], psB
                        for ws in range(cfg.n_ws):
                            w0 = ws * cfg.w_sl
                            nw = min(cfg.w_sl, Wc - w0)
                            nc.tensor.matmul(
                                pst[ws][32 * g32:32 * g32 + 32, :],
                                selt[:, 32 * v:32 * v + 32],
                                p[:, w0:w0 + nw, :],
                                start=(v == 0), stop=(v == nv - 1),
                                tile_position=(0, 32 * g32),
                            )
                        if i % 4 == 3 and bq:
                            bq.pop(0)()
                    if j == t - 1:
                        while bq:
                            bq.pop(0)()
                    for ws in range(cfg.n_ws):
                        w0 = ws * cfg.w_sl
                        nw = min(cfg.w_sl, Wc - w0)
                        nfree = nw * D
                        stA = spool.tile([128, nfree], BF16, tag="stA")
                        stB = spool.tile([RB, nfree], BF16, tag="stB")
                        nc.scalar.activation(
                            stA[:], psA[ws][:, 0:nfree], AF.Copy,
                            accum_out=accA[:, drain_i:drain_i + 1])
                        nc.scalar.activation(
                            stB[:], psB[ws][0:RB, 0:nfree], AF.Copy,
                            accum_out=accB[:, drain_i:drain_i + 1])
                        nc.sync.dma_start(
                            spill[0:128, hb0 + j, w0:w0 + nw, :], stA[:])
                        nc.sync.dma_start(
                            spill[128:RSP, hb0 + j, w0:w0 + nw, :], stB[:])
                        drain_i += 1

            # ---- B-path squeeze: per-shift spatial sums, then sum over d ----
            for ki in range(NB):
                nc.scalar.activation(
                    scrB[:], corrB[:, ki, :, :], AF.Copy,
                    accum_out=pBd[:, ki:ki + 1])
            psq = ps.tile([1, NB], FP32, tag="psA0", padded_shape=[128, 512])
            nc.tensor.matmul(psq[:], onesct[:], pBd[:], start=True, stop=True)
            sBt = cpool.tile([1, NB], FP32)
            nc.scalar.activation(sBt[:], psq[:], AF.Copy)

            # ---- last-chunk partials + final allreduce + gate MLP ----
            pA = cpool.tile([128, 1], FP32)
            pB = cpool.tile([RB, 1], FP32)
            scrA = cpool.tile([128, n_drain - early_drains], FP32)
            scrB2 = cpool.tile([RB, n_drain - early_drains], FP32)
            nc.scalar.activation(scrA[:], accA[:, early_drains:n_drain],
                                 AF.Copy, accum_out=pA[:])
            nc.scalar.activation(scrB2[:], accB[:, early_drains:n_drain],
                                 AF.Copy, accum_out=pB[:])
            pAg1 = cpool.tile([128, 1], FP32)
            pBg1 = cpool.tile([RB, 1], FP32)
            nc.sync.dma_start(pAg1[:], cc1_out[0:128, :])
            nc.sync.dma_start(pBg1[:], cc1_out[128:128 + RB, :])
            ccA = cpool.tile([128, 1], FP32)
            ccB = cpool.tile([RB, 1], FP32)
            nc.scalar.activation(ccA[:], pAg1[:], AF.Identity,
                                 bias=pA[:], scale=1.0 / N_CORES)
            nc.scalar.activation(ccB[:], pBg1[:], AF.Identity,
                                 bias=pB[:], scale=1.0 / N_CORES)
            cc_in = dram.tile([CCR, 1], FP32)
            cc_out = dram.tile([CCR, 1], FP32)
            nc.sync.dma_start(cc_in[0:128, :], ccA[:])
            nc.sync.dma_start(cc_in[128:128 + RB, :], ccB[:])
            nc.sync.dma_start(cc_in[128 + RB:CCR, :], sBt[:])
            nc.gpsimd.collective_compute(
                "AllReduce", ALU.add,
                replica_groups=[list(range(N_CORES))],
                ins=[cc_in[:].opt()],
                outs=[cc_out[:].opt()],
            )
            pAg = cpool.tile([128, 1], FP32)
            pBg = cpool.tile([RB + NB, 1], FP32)
            nc.sync.dma_start(pAg[:], cc_out[0:128, :])
            nc.sync.dma_start(pBg[:], cc_out[128:CCR, :])

            hps = ps.tile([MID, 1], FP32, tag="psA1", padded_shape=[128, 512])
            nc.tensor.matmul(hps[:], w0at[:], pAg[:], start=True, stop=False)
            nc.tensor.matmul(hps[:], w0bt[:], pBg[:], start=False, stop=True)
            hvec = cpool.tile([MID, 1], FP32)
            nc.scalar.activation(hvec[:], hps[:], AF.Relu, bias=b0t[:], scale=1.0)
            gpsA = ps.tile([128, 1], FP32, tag="psA2", padded_shape=[128, 512])
            gpsB = ps.tile([RB, 1], FP32, tag="psB0", padded_shape=[128, 512])
            nc.tensor.matmul(gpsA[:], w1at[:], hvec[:], start=True, stop=True)
            nc.tensor.matmul(gpsB[:], w1bt[:], hvec[:], start=True, stop=True)
            gA = cpool.tile([128, 1], FP32)
            gB = cpool.tile([RB, 1], FP32)
            nc.scalar.activation(gA[:], gpsA[:], AF.Sigmoid, bias=b1at[:], scale=1.0)
            nc.scalar.activation(gB[:], gpsB[:], AF.Sigmoid, bias=b1bt[:], scale=1.0)

            # B-gates: [1,NB] row = sigmoid(hvec.T @ w15 + b15), broadcast to d
            g5ps = ps.tile([1, NB], FP32, tag="psB1", padded_shape=[128, 512])
            nc.tensor.matmul(g5ps[:], hvec[:], w15t[:], start=True, stop=True)
            g5s = cpool.tile([1, NB], FP32)
            nc.vector.tensor_tensor(g5s[:], g5ps[:], b15t[:], ALU.add)
            g5t = cpool.tile([1, NB], FP32)
            nc.scalar.activation(g5t[:], g5s[:], AF.Sigmoid)
            gbps = ps.tile([D, NB], FP32, tag="psB2", padded_shape=[128, 512])
            nc.tensor.matmul(gbps[:], onesrt[:], g5t[:], start=True, stop=True)
            gbc = cpool.tile([D, NB], FP32)
            nc.scalar.activation(gbc[:], gbps[:], AF.Copy)

            # ---- phase 2: gated writeout ----
            for ki in range(NB):
                ob = p2pool.tile([D, H, Wc], BF16, tag="p2b", bufs=2)
                nc.vector.tensor_scalar(ob[:], corrB[:, ki, :, :],
                                        gbc[:, ki:ki + 1], None, ALU.mult)
                nc.scalar.dma_start(outb_d[:, ki, :, :], ob[:])
            assert HB % 2 == 0
            for hb in range(0, HB, 2):
                stA2i = p2pool.tile([128, 2, Wc, D], BF16, tag="p2ai", bufs=3)
                stA2o = p2pool.tile([128, 2, Wc, D], BF16, tag="p2ao", bufs=2)
                nc.sync.dma_start(stA2i[:], spill[0:128, hb:hb + 2, :, :])
                nc.vector.tensor_scalar(stA2o[:], stA2i[:], gA[:], None, ALU.mult)
                nc.scalar.dma_start(out_d[0:128, hb:hb + 2, :, :], stA2o[:])
                stB2i = p2pool.tile([RB, 2, Wc, D], BF16, tag="p2bi", bufs=3)
                stB2o = p2pool.tile([RB, 2, Wc, D], BF16, tag="p2bo", bufs=2)
                nc.sync.dma_start(stB2i[:], spill[128:RSP, hb:hb + 2, :, :])
                nc.vector.tensor_scalar(stB2o[:], stB2i[:], gB[:], None, ALU.mult)
                nc.scalar.dma_start(out_d[128:RSP, hb:hb + 2, :, :], stB2o[:])

    nc.compile()
    return nc


# ---------------- host-side prep / assembly ----------------

def make_gate_consts(w0, b0, w1, b1, cfg: Cfg):
    norm = 1.0 / (cfg.W * cfg.H * cfg.D)
    sel = np.zeros((128, 128), dtype=np.float32)
    for v in range(4):
        for c in range(C):
            for h8 in range(H8):
                sel[c * H8 + h8, 32 * v + 8 * v + h8] = 1.0 / 16
    w0 = np.asarray(w0, dtype=np.float32)
    w1 = np.asarray(w1, dtype=np.float32)
    b1 = np.asarray(b1, dtype=np.float32)
    w0a = np.zeros((128, MID), dtype=np.float32)
    w0b = np.zeros((RB + NB, MID), dtype=np.float32)
    w1ra = np.zeros((MID, 128), dtype=np.float32)
    w1rb = np.zeros((MID, RB), dtype=np.float32)
    b1ra = np.zeros((128, 1), dtype=np.float32)
    b1rb = np.zeros((RB, 1), dtype=np.float32)
    for k in PE_SHIFTS:
        for h8 in range(H8):
            r = _row_of(k, h8)
            if r < 128:
                w0a[r, :] = w0[:, k] * norm
                w1ra[:, r] = w1[k, :]
                b1ra[r, 0] = b1[k]
            else:
                w0b[r - 128, :] = w0[:, k] * norm
                w1rb[:, r - 128] = w1[k, :]
                b1rb[r - 128, 0] = b1[k]
    w15 = np.zeros((MID, NB), dtype=np.float32)
    b15 = np.zeros((1, NB), dtype=np.float32)
    for ki, k in enumerate(B_SET):
        w0b[RB + ki, :] = w0[:, k] * norm
        w15[:, ki] = w1[k, :]
        b15[0, ki] = b1[k]
    return {
        "selmats": sel.astype(ml_dtypes.bfloat16),
        "w0a": w0a, "w0b": w0b, "w1ra": w1ra, "w1rb": w1rb,
        "w15": w15, "b15": b15,
        "b0c": np.asarray(b0, dtype=np.float32).reshape(MID, 1),
        "b1ra": b1ra, "b1rb": b1rb,
        "ones_c": np.ones((cfg.D, 1), dtype=np.float32),
        "ones_r": np.ones((1, cfg.D), dtype=np.float32),
    }


def _fold(a, HB):
    # [C, w, H, D'] -> [(c h8), hblk, w, d]
    Cc, ww, hh, dd = a.shape
    a = a.reshape(Cc, ww, H8, HB, dd)
    a = np.ascontiguousarray(a.transpose(0, 2, 3, 1, 4))
    return a.reshape(C * H8, HB, ww, dd)


def _fold_halo(a, HB):
    # [C, w, Hp=H+2, D'] (padded h) -> [(c h8), HB+2, w, d] with h-halo rows
    Cc, ww, hp, dd = a.shape
    out = np.empty((Cc, H8, HB + 2, ww, dd), dtype=a.dtype)
    for h8 in range(H8):
        out[:, h8] = a[:, :, h8 * HB:h8 * HB + HB + 2, :].transpose(0, 2, 1, 3)
    return out.reshape(Cc * H8, HB + 2, ww, dd)


def make_inputs_per_core(x_1, x_2, w0, b0, w1, b1, cfg: Cfg):
    """x_1/x_2: [1, C, W, H, D] float32 -> list of per-core input dicts."""
    W, H, D = cfg.W, cfg.H, cfg.D
    Wc, HB = cfg.Wc, cfg.HB
    x1 = np.asarray(x_1)[0].astype(ml_dtypes.bfloat16)      # [C, W, H, D]
    x2 = np.asarray(x_2)[0].astype(ml_dtypes.bfloat16)
    x2p = np.zeros((C, W + 2, H + 2, D + 2), dtype=ml_dtypes.bfloat16)
    x2p[:, 1:W + 1, 1:H + 1, 1:D + 1] = x2

    consts = make_gate_consts(w0, b0, w1, b1, cfg)
    in_maps = []
    for ci in range(N_CORES):
        ws = ci * Wc
        m = dict(consts)
        m["x1"] = _fold(x1[:, ws:ws + Wc, :, :], HB)
        m["x2a"] = _fold_halo(x2p[:, ws:ws + Wc + 2, :, :], HB)
        m["x1b"] = np.ascontiguousarray(
            x1[:, ws:ws + Wc, :, :].transpose(3, 0, 2, 1)
        ).astype(np.float32).__mul__(1.0 / 16).astype(ml_dtypes.bfloat16)
        m["x2b"] = np.ascontiguousarray(
            x2p[:, ws:ws + Wc + 2, :, 1:1 + D].transpose(3, 0, 2, 1))
        in_maps.append(m)
    return in_maps


def assemble_output(results, cfg: Cfg):
    W, H, D = cfg.W, cfg.H, cfg.D
    Wc, HB = cfg.Wc, cfg.HB
    out = np.empty((W, H, D, K), dtype=np.float32)
    rows = np.empty((len(PE_SHIFTS), H8), dtype=np.int64)
    for i, k in enumerate(PE_SHIFTS):
        for h8 in range(H8):
            rows[i, h8] = _row_of(k, h8)
    for ci, r in enumerate(results):
        o = np.asarray(r["out"]).reshape(128 + RB, HB, Wc, D)
        core = o[rows]                        # [npe, H8, HB, Wc, D]
        core = core.transpose(3, 1, 2, 4, 0)  # [Wc, H8, HB, D, npe]
        core = core.reshape(Wc, H, D, len(PE_SHIFTS))
        for i, k in enumerate(PE_SHIFTS):
            out[ci * Wc:(ci + 1) * Wc, :, :, k] = core[..., i]
        ob = np.asarray(r["outb"]).reshape(D, NB, H, Wc)
        for ki, k in enumerate(B_SET):
            out[ci * Wc:(ci + 1) * Wc, :, :, k] = ob[:, ki].transpose(2, 1, 0)
    return out[None]


_CACHE = {}
TRACE = False           # test harness can set kernel.TRACE = True


def kernel(x_1, x_2, w0, b0, w1, b1):
    cfg = Cfg()
    if "nc" not in _CACHE:
        _CACHE["nc"] = build_nc(cfg)
    nc = _CACHE["nc"]
    in_maps = make_inputs_per_core(x_1, x_2, w0, b0, w1, b1, cfg)
    last_exc = None
    for _attempt in range(3):
        try:
            res = run_bass_kernel_spmd(nc, in_maps,
                                       core_ids=list(range(N_CORES)),
                                       trace=TRACE)
            break
        except Exception as e:  # transient NRT device errors: retry
            last_exc = e
    else:
        raise last_exc
    _CACHE["last_res"] = res
    return assemble_output(res.results, cfg)


# revision 11
# speedup vs baseline: 1.2183x; 1.0785x over previous
"""Trainium2 Bass kernel for shifted-window correlation (27 shifts) + SE gate.

Reference computation (shapes hardcoded; B=1, C=16, W=80, H=96, D=112):
  corr[w,h,d,k] = mean_c x1[c,w,h,d] * x2[c, w+sx, h+sy, d+sz]   (zero-padded)
  s = mean_{w,h,d} corr;  g = sigmoid(w1 @ relu(w0 @ s + b0) + b1)
  out = corr * g

Strategy (8 cores, W sharded 10/core):
  - SBUF partition dim = (c:16, h8:8) where h8 = h // (H/8).
  - Shifts: sy via 3 h-shifted DMA loads of x2; sx as free-dim w offset
    (w halo in the loaded window); sz via even/odd d-parity loads so all
    bf16 tensor_tensor operands stay 4-byte aligned (DVE 2x mode).
  - Products on DVE (bf16, 2x); channel reduction on the PE via a fixed
    block-diagonal selection matmul (1/16 entries) accumulating 4 shifts
    per 32-partition PSUM column group; ACT drains PSUM -> SBUF with
    accum_out capturing squeeze partial sums; ungated corr spilled to DRAM.
  - Squeeze: per-core partials -> AllReduce (216 floats) -> on-device gate
    MLP (two tiny matmuls + relu/sigmoid, replicated per partition row).
  - Phase 2 re-reads spilled corr (prefetch hides in collective latency),
    multiplies by per-partition gate, writes out. Host reassembles +
    transposes to [1, W, H, D, 27].
"""

import sys
import types

import numpy as np
import ml_dtypes


def _install_ntff_hook_shim():
    """agent image's antenv lacks axon_hooks; needed only for trace=True."""
    if "antenv.axon_hooks" in sys.modules:
        return
    try:
        import antenv
        from trn_agent_boot.trn_boot import _ntff_profile_via_ctypes

        hook = _ntff_profile_via_ctypes("/opt/axon/libaxon_pjrt.so")
        mod = types.ModuleType("antenv.axon_hooks")
        ref = {"h": hook}
        mod.get_axon_ntff_profile_hook = lambda: ref["h"]
        mod.set_axon_ntff_profile_hook = lambda h: ref.__setitem__("h", h)
        sys.modules["antenv.axon_hooks"] = mod
        antenv.axon_hooks = mod
    except Exception:
        pass


_install_ntff_hook_shim()

import concourse.bacc as bacc  # noqa: E402
import concourse.tile as tile  # noqa: E402
import concourse.mybir as mybir  # noqa: E402
from concourse.bass_utils import run_bass_kernel_spmd  # noqa: E402

BF16 = mybir.dt.bfloat16
FP32 = mybir.dt.float32
AF = mybir.ActivationFunctionType
ALU = mybir.AluOpType

N_CORES = 8
C = 16
H8 = 8          # partition sub-dim over h
K = 27
MID = 6


class Cfg:
    def __init__(self, W=80, H=96, D=112, hblk_t=3, w_sl=4):
        assert H % H8 == 0
        self.W, self.H, self.D = W, H, D
        self.Wc = W // N_CORES          # w columns per core
        self.HB = H // H8               # hblk extent (free dim)
        assert self.HB % hblk_t == 0
        self.hblk_t = hblk_t            # hblk rows per chunk
        self.n_chunks = self.HB // hblk_t
        self.w_sl = w_sl                # w per matmul slice
        self.n_ws = (self.Wc + w_sl - 1) // w_sl
        self.De = D + 2                 # odd-copy d extent


# shift order matches reference: k = dx*9 + dy*3 + dz, s* = d*-1
SHIFTS = [(dx - 1, dy - 1, dz - 1)
          for dx in range(3) for dy in range(3) for dz in range(3)]


def _row_of(k, h8):
    """PSUM/spill partition row for (k, h8). Tile A: k 0..15, tile B: 16..26."""
    kk = k if k < 16 else k - 16
    base = 0 if k < 16 else 128
    return base + 32 * (kk // 4) + 8 * (kk % 4) + h8


def build_nc(cfg: Cfg):
    nc = bacc.Bacc("TRN2", target_bir_lowering=False, debug=False,
                   num_devices=N_CORES)
    HB, Wc, D, De = cfg.HB, cfg.Wc, cfg.D, cfg.De

    x1_d = nc.dram_tensor("x1", [128, HB, Wc, D], BF16, kind="ExternalInput")
    x2_d = {}
    for sy in (-1, 0, 1):
        x2_d[(sy, 0)] = nc.dram_tensor(f"x2_s{sy+1}_e", [128, HB, Wc + 2, D],
                                       BF16, kind="ExternalInput")
        x2_d[(sy, 1)] = nc.dram_tensor(f"x2_s{sy+1}_o", [128, HB, Wc + 2, De],
                                       BF16, kind="ExternalInput")
    sel_d = nc.dram_tensor("selmats", [128, 128], BF16, kind="ExternalInput")
    w0a_d = nc.dram_tensor("w0a", [128, MID], FP32, kind="ExternalInput")
    w0b_d = nc.dram_tensor("w0b", [88, MID], FP32, kind="ExternalInput")
    w1a_d = nc.dram_tensor("w1ra", [MID, 128], FP32, kind="ExternalInput")
    w1b_d = nc.dram_tensor("w1rb", [MID, 88], FP32, kind="ExternalInput")
    b0_d = nc.dram_tensor("b0c", [MID, 1], FP32, kind="ExternalInput")
    b1a_d = nc.dram_tensor("b1ra", [128, 1], FP32, kind="ExternalInput")
    b1b_d = nc.dram_tensor("b1rb", [88, 1], FP32, kind="ExternalInput")
    out_d = nc.dram_tensor("out", [216, HB, Wc, D], BF16, kind="ExternalOutput")

    n_drain = cfg.n_chunks * cfg.hblk_t * cfg.n_ws

    with tile.TileContext(nc) as tc:
        with (
            tc.tile_pool(name="const", bufs=1) as cpool,
            tc.tile_pool(name="ps", bufs=1, space="PSUM") as ps,
            tc.tile_pool(name="dram", bufs=1, space="DRAM") as dram,
            tc.tile_pool(name="x2p", bufs=2) as x2pool,
            tc.tile_pool(name="pp", bufs=4) as ppool,
            tc.tile_pool(name="stage", bufs=4) as spool,
            tc.tile_pool(name="ph2", bufs=2) as p2pool,
        ):
            # resident constants / inputs
            x1t = cpool.tile([128, HB, Wc, D], BF16)
            selt = cpool.tile([128, 128], BF16)
            w0at = cpool.tile([128, MID], FP32)
            w0bt = cpool.tile([88, MID], FP32)
            w1at = cpool.tile([MID, 128], FP32)
            w1bt = cpool.tile([MID, 88], FP32)
            b0t = cpool.tile([MID, 1], FP32)
            b1at = cpool.tile([128, 1], FP32)
            b1bt = cpool.tile([88, 1], FP32)
            accA = cpool.tile([128, n_drain], FP32)
            accB = cpool.tile([88, n_drain], FP32)

            nc.sync.dma_start(selt[:], sel_d[:])
            nc.sync.dma_start(w0at[:], w0a_d[:])
            nc.sync.dma_start(w0bt[:], w0b_d[:])
            nc.sync.dma_start(w1at[:], w1a_d[:])
            nc.sync.dma_start(w1bt[:], w1b_d[:])
            nc.sync.dma_start(b0t[:], b0_d[:])
            nc.sync.dma_start(b1at[:], b1a_d[:])
            nc.sync.dma_start(b1bt[:], b1b_d[:])

            spill = dram.tile([216, HB, Wc, D], BF16)

            # Warm-up collective: absorbs cross-core launch skew and CC
            # firmware setup while phase-1 compute runs, so the real
            # allreduce at the end only pays ~10-20us marginal latency.
            warm_in = dram.tile([MID, 1], FP32)
            warm_out = dram.tile([MID, 1], FP32)
            nc.sync.dma_start(warm_in[:], b0_d[:])
            nc.gpsimd.collective_compute(
                "AllReduce", ALU.add,
                replica_groups=[list(range(N_CORES))],
                ins=[warm_in[:].opt()],
                outs=[warm_out[:].opt()],
            )

            # partial-sum allreduce is split: chunks 0..n-2 reduced early so
            # that collective's latency hides under the last chunk's compute.
            early_drains = (cfg.n_chunks - 1) * cfg.hblk_t * cfg.n_ws
            pA1 = cpool.tile([128, 1], FP32)
            pB1 = cpool.tile([88, 1], FP32)
            cc1_in = dram.tile([216, 1], FP32)
            cc1_out = dram.tile([216, 1], FP32)

            drain_i = 0
            for ch in range(cfg.n_chunks):
                hb0 = ch * cfg.hblk_t

                if ch == cfg.n_chunks - 1:
                    # early allreduce over chunks 0..n-2 partials
                    nc.vector.tensor_reduce(
                        pA1[:], accA[:, 0:early_drains],
                        mybir.AxisListType.X, ALU.add)
                    nc.vector.tensor_reduce(
                        pB1[:], accB[:, 0:early_drains],
                        mybir.AxisListType.X, ALU.add)
                    nc.sync.dma_start(cc1_in[0:128, :], pA1[:])
                    nc.sync.dma_start(cc1_in[128:216, :], pB1[:])
                    nc.gpsimd.collective_compute(
                        "AllReduce", ALU.add,
                        replica_groups=[list(range(N_CORES))],
                        ins=[cc1_in[:].opt()],
                        outs=[cc1_out[:].opt()],
                    )

                nc.sync.dma_start(x1t[:, hb0:hb0 + cfg.hblk_t, :, :],
                                  x1_d[:, hb0:hb0 + cfg.hblk_t, :, :])

                for j in range(cfg.hblk_t):
                    # per-row x2 windows so first products start early
                    x2t = {}
                    for sy in (-1, 0, 1):
                        for par in (0, 1):
                            dd = D if par == 0 else De
                            t = x2pool.tile([128, Wc + 2, dd], BF16,
                                            tag=f"x2_{sy}_{par}", bufs=5,
                                            name=f"x2_{sy}_{par}")
                            nc.sync.dma_start(
                                t[:], x2_d[(sy, par)][:, hb0 + j, :, :])
                            x2t[(sy, par)] = t
                    psA = [ps.tile([128, min(cfg.w_sl, Wc - ws * cfg.w_sl) * D],
                                   FP32, tag=f"psA{ws}", name=f"psA{ws}",
                                   padded_shape=[128, 512])
                           for ws in range(cfg.n_ws)]
                    psB = [ps.tile([128, min(cfg.w_sl, Wc - ws * cfg.w_sl) * D],
                                   FP32, tag=f"psB{ws}", name=f"psB{ws}",
                                   padded_shape=[128, 512])
                           for ws in range(cfg.n_ws)]
                    for k, (sx, sy, sz) in enumerate(SHIFTS):
                        par = 0 if sz == 0 else 1
                        doff = 0 if sz == 0 else sz + 1
                        src = x2t[(sy, par)]
                        p = ppool.tile([128, Wc, D], BF16, tag="P", bufs=6)
                        nc.vector.tensor_tensor(
                            p[:],
                            x1t[:, hb0 + j, :, :],
                            src[:, 1 + sx:1 + sx + Wc, doff:doff + D],
                            ALU.mult,
                        )
                        kk = k if k < 16 else k - 16
                        g32, v = kk // 4, kk % 4
                        nv = 4 if (k < 16 or g32 < 2) else 3
                        for ws in range(cfg.n_ws):
                            w0 = ws * cfg.w_sl
                            nw = min(cfg.w_sl, Wc - w0)
                            pst = psA[ws] if k < 16 else psB[ws]
                            nc.tensor.matmul(
                                pst[32 * g32:32 * g32 + 32, :],
                                selt[:, 32 * v:32 * v + 32],
                                p[:, w0:w0 + nw, :],
                                start=(v == 0), stop=(v == nv - 1),
                                tile_position=(0, 32 * g32),
                            )
                    for ws in range(cfg.n_ws):
                        w0 = ws * cfg.w_sl
                        nw = min(cfg.w_sl, Wc - w0)
                        nfree = nw * D
                        stA = spool.tile([128, nfree], BF16, tag="stA")
                        stB = spool.tile([88, nfree], BF16, tag="stB")
                        nc.scalar.activation(
                            stA[:], psA[ws][:, 0:nfree], AF.Copy,
                            accum_out=accA[:, drain_i:drain_i + 1])
                        nc.scalar.activation(
                            stB[:], psB[ws][0:88, 0:nfree], AF.Copy,
                            accum_out=accB[:, drain_i:drain_i + 1])
                        nc.sync.dma_start(
                            spill[0:128, hb0 + j, w0:w0 + nw, :], stA[:])
                        nc.sync.dma_start(
                            spill[128:216, hb0 + j, w0:w0 + nw, :], stB[:])
                        drain_i += 1

            # ---- last-chunk partials + final allreduce + gate ----
            # partials of the last chunk reduced on ACT (idle once drains
            # finish; Vector is still busy with products), cc1's global
            # result folded into cc2's input scaled by 1/N_CORES.
            pA = cpool.tile([128, 1], FP32)
            pB = cpool.tile([88, 1], FP32)
            scrA = cpool.tile([128, n_drain - early_drains], FP32)
            scrB = cpool.tile([88, n_drain - early_drains], FP32)
            nc.scalar.activation(scrA[:], accA[:, early_drains:n_drain],
                                 AF.Copy, accum_out=pA[:])
            nc.scalar.activation(scrB[:], accB[:, early_drains:n_drain],
                                 AF.Copy, accum_out=pB[:])
            pAg1 = cpool.tile([128, 1], FP32)
            pBg1 = cpool.tile([88, 1], FP32)
            nc.sync.dma_start(pAg1[:], cc1_out[0:128, :])
            nc.sync.dma_start(pBg1[:], cc1_out[128:216, :])
            ccA = cpool.tile([128, 1], FP32)
            ccB = cpool.tile([88, 1], FP32)
            nc.scalar.activation(ccA[:], pAg1[:], AF.Identity,
                                 bias=pA[:], scale=1.0 / N_CORES)
            nc.scalar.activation(ccB[:], pBg1[:], AF.Identity,
                                 bias=pB[:], scale=1.0 / N_CORES)
            cc_in = dram.tile([216, 1], FP32)
            cc_out = dram.tile([216, 1], FP32)
            nc.sync.dma_start(cc_in[0:128, :], ccA[:])
            nc.sync.dma_start(cc_in[128:216, :], ccB[:])
            nc.gpsimd.collective_compute(
                "AllReduce", ALU.add,
                replica_groups=[list(range(N_CORES))],
                ins=[cc_in[:].opt()],
                outs=[cc_out[:].opt()],
            )
            pAg = cpool.tile([128, 1], FP32)
            pBg = cpool.tile([88, 1], FP32)
            nc.sync.dma_start(pAg[:], cc_out[0:128, :])
            nc.sync.dma_start(pBg[:], cc_out[128:216, :])

            hps = ps.tile([MID, 1], FP32, tag="psA0", padded_shape=[128, 512])
            nc.tensor.matmul(hps[:], w0at[:], pAg[:], start=True, stop=False)
            nc.tensor.matmul(hps[:], w0bt[:], pBg[:], start=False, stop=True)
            hvec = cpool.tile([MID, 1], FP32)
            nc.scalar.activation(hvec[:], hps[:], AF.Relu, bias=b0t[:], scale=1.0)
            gpsA = ps.tile([128, 1], FP32, tag="psA1", padded_shape=[128, 512])
            gpsB = ps.tile([88, 1], FP32, tag="psA2", padded_shape=[128, 512])
            nc.tensor.matmul(gpsA[:], w1at[:], hvec[:], start=True, stop=True)
            nc.tensor.matmul(gpsB[:], w1bt[:], hvec[:], start=True, stop=True)
            gA = cpool.tile([128, 1], FP32)
            gB = cpool.tile([88, 1], FP32)
            nc.scalar.activation(gA[:], gpsA[:], AF.Sigmoid, bias=b1at[:], scale=1.0)
            nc.scalar.activation(gB[:], gpsB[:], AF.Sigmoid, bias=b1bt[:], scale=1.0)

            # ---- phase 2: gated writeout (per hblk row; reads prefetch
            # into the collective's latency window) ----
            assert HB % 2 == 0
            for hb in range(0, HB, 2):
                stA2i = p2pool.tile([128, 2, Wc, D], BF16, tag="p2ai", bufs=4)
                stA2o = p2pool.tile([128, 2, Wc, D], BF16, tag="p2ao", bufs=3)
                nc.sync.dma_start(stA2i[:], spill[0:128, hb:hb + 2, :, :])
                nc.vector.tensor_scalar(stA2o[:], stA2i[:], gA[:], None, ALU.mult)
                nc.scalar.dma_start(out_d[0:128, hb:hb + 2, :, :], stA2o[:])
                stB2i = p2pool.tile([88, 2, Wc, D], BF16, tag="p2bi", bufs=4)
                stB2o = p2pool.tile([88, 2, Wc, D], BF16, tag="p2bo", bufs=3)
                nc.sync.dma_start(stB2i[:], spill[128:216, hb:hb + 2, :, :])
                nc.vector.tensor_scalar(stB2o[:], stB2i[:], gB[:], None, ALU.mult)
                nc.scalar.dma_start(out_d[128:216, hb:hb + 2, :, :], stB2o[:])

    nc.compile()
    return nc


# ---------------- host-side prep / assembly ----------------

def make_gate_consts(w0, b0, w1, b1, cfg: Cfg):
    norm = 1.0 / (cfg.W * cfg.H * cfg.D)
    sel = np.zeros((128, 128), dtype=np.float32)
    for v in range(4):
        for c in range(C):
            for h8 in range(H8):
                sel[c * H8 + h8, 32 * v + 8 * v + h8] = 1.0 / 16
    w0 = np.asarray(w0, dtype=np.float32)
    w1 = np.asarray(w1, dtype=np.float32)
    b1 = np.asarray(b1, dtype=np.float32)
    w0a = np.zeros((128, MID), dtype=np.float32)
    w0b = np.zeros((88, MID), dtype=np.float32)
    w1ra = np.zeros((MID, 128), dtype=np.float32)
    w1rb = np.zeros((MID, 88), dtype=np.float32)
    b1ra = np.zeros((128, 1), dtype=np.float32)
    b1rb = np.zeros((88, 1), dtype=np.float32)
    for k in range(K):
        for h8 in range(H8):
            r = _row_of(k, h8)
            if k < 16:
                w0a[r, :] = w0[:, k] * norm
                w1ra[:, r] = w1[k, :]
                b1ra[r, 0] = b1[k]
            else:
                w0b[r - 128, :] = w0[:, k] * norm
                w1rb[:, r - 128] = w1[k, :]
                b1rb[r - 128, 0] = b1[k]
    return {
        "selmats": sel.astype(ml_dtypes.bfloat16),
        "w0a": w0a, "w0b": w0b, "w1ra": w1ra, "w1rb": w1rb,
        "b0c": np.asarray(b0, dtype=np.float32).reshape(MID, 1),
        "b1ra": b1ra, "b1rb": b1rb,
    }


def _fold(a, HB):
    # [C, w, H, D'] -> [(c h8), hblk, w, d]
    Cc, ww, hh, dd = a.shape
    a = a.reshape(Cc, ww, H8, HB, dd)
    a = np.ascontiguousarray(a.transpose(0, 2, 3, 1, 4))
    return a.reshape(C * H8, HB, ww, dd)


def make_inputs_per_core(x_1, x_2, w0, b0, w1, b1, cfg: Cfg):
    """x_1/x_2: [1, C, W, H, D] float32 -> list of per-core input dicts."""
    W, H, D, De = cfg.W, cfg.H, cfg.D, cfg.De
    Wc, HB = cfg.Wc, cfg.HB
    x1 = np.asarray(x_1)[0].transpose(0, 1, 2, 3)  # [C, W, H, D]
    x1 = x1.astype(ml_dtypes.bfloat16)
    x2 = np.asarray(x_2)[0].astype(ml_dtypes.bfloat16)
    # padded x2: w +-1, h +-1, d in [-1, D+1)
    x2p = np.zeros((C, W + 2, H + 2, D + 2), dtype=ml_dtypes.bfloat16)
    x2p[:, 1:W + 1, 1:H + 1, 1:D + 1] = x2

    consts = make_gate_consts(w0, b0, w1, b1, cfg)
    in_maps = []
    for ci in range(N_CORES):
        ws = ci * Wc
        m = dict(consts)
        m["x1"] = _fold(x1[:, ws:ws + Wc, :, :], HB)
        for sy in (-1, 0, 1):
            hsl = slice(1 + sy, 1 + sy + H)
            wsl = slice(ws, ws + Wc + 2)
            m[f"x2_s{sy+1}_e"] = _fold(x2p[:, wsl, hsl, 1:1 + D], HB)
            m[f"x2_s{sy+1}_o"] = _fold(x2p[:, wsl, hsl, 0:De], HB)
        in_maps.append(m)
    return in_maps


def assemble_output(results, cfg: Cfg):
    W, H, D = cfg.W, cfg.H, cfg.D
    Wc, HB = cfg.Wc, cfg.HB
    rows = np.empty((K, H8), dtype=np.int64)
    for k in range(K):
        for h8 in range(H8):
            rows[k, h8] = _row_of(k, h8)
    out = np.empty((W, H, D, K), dtype=np.float32)
    for ci, r in enumerate(results):
        o = np.asarray(r["out"]).reshape(216, HB, Wc, D)
        core = o[rows]                        # [K, H8, HB, Wc, D]
        core = core.transpose(3, 1, 2, 4, 0)  # [Wc, H8, HB, D, K]
        out[ci * Wc:(ci + 1) * Wc] = core.reshape(Wc, H, D, K)
    return out[None]


_CACHE = {}
TRACE = False           # test harness can set kernel.TRACE = True


def kernel(x_1, x_2, w0, b0, w1, b1):
    cfg = Cfg()
    if "nc" not in _CACHE:
        _CACHE["nc"] = build_nc(cfg)
    nc = _CACHE["nc"]
    in_maps = make_inputs_per_core(x_1, x_2, w0, b0, w1, b1, cfg)
    last_exc = None
    for _attempt in range(3):
        try:
            res = run_bass_kernel_spmd(nc, in_maps,
                                       core_ids=list(range(N_CORES)),
                                       trace=TRACE)
            break
        except Exception as e:  # transient NRT device errors: retry
            last_exc = e
    else:
        raise last_exc
    _CACHE["last_res"] = res
    return assemble_output(res.results, cfg)
